# revision 5
# baseline (speedup 1.0000x reference)
"""BinaryDense forward kernel for Trainium2 (8 NeuronCores, data-parallel).

Computes y = x @ w_bin + bias where w_bin is the stochastic binarization
({-1,+1}) of a 128x128 weight matrix (fixed bernoulli key 42, matching the
jax reference bit-exactly; computed on host since it is tiny).

Sharding: x [2097152, 128] f32 is split along M into 8 shards of
[262144, 128], one per NeuronCore; w_bin and bias are replicated.

Per-core kernel (all fp32, exact):
  - SBUF partition p holds a contiguous block of 2048 rows of the shard, so
    every HBM<->SBUF DMA moves 16 KiB contiguous per partition per chunk.
  - Each [128,128] row-tile is transposed on the TensorEngine (transpose
    mode, exact pass-through) into PSUM, evicted to SBUF by the ScalarE,
    then used as the stationary operand of a fp32 matmul against the
    replicated w_bin; y lands in PSUM with M on partitions and is evicted
    (+bias, pre-tiled on host to [128,512]) by the VectorE, then DMA'd out.
"""

import numpy as np

P = 128  # partitions
K = 128  # contraction dim
N = 128  # output features
M_TOTAL = 2097152
NCORES = 8
M_LOCAL = M_TOTAL // NCORES  # 262144
ROWS_PER_PART = M_LOCAL // P  # 2048 rows of x per SBUF partition
T = 32  # row-tiles per chunk (each tile = 128 rows spread across partitions)
NCHUNK = ROWS_PER_PART // T  # 64 chunks per core
GROUP = 4  # tiles per PSUM bank
NG = T // GROUP  # groups per chunk
CH = T * K  # free elements per chunk per partition

_cache = {}


def _binarize_weight(weight: np.ndarray, is_training) -> np.ndarray:
    """Reproduce the reference's stochastic binarization bit-exactly."""
    training = bool(np.asarray(is_training).item())
    if not training:
        return np.where(weight > 0, 1.0, -1.0).astype(np.float32)
    import jax
    import jax.numpy as jnp

    with jax.default_device(jax.devices("cpu")[0]):
        w = jnp.asarray(weight, dtype=jnp.float32)
        prob_pos = jnp.clip((w + 1.0) / 2.0, 0.0, 1.0)
        bern = jax.random.bernoulli(jax.random.key(42), prob_pos, w.shape)
        w_bin = jnp.where(bern, 1.0, -1.0).astype(jnp.float32)
        return np.asarray(w_bin)


def _build(reps: int = 1, mode: str = "full"):
    key = ("nc", reps, mode)
    if key in _cache:
        return _cache[key]

    import concourse.tile as tile
    from concourse import bacc, mybir
    from concourse.masks import make_identity
    from contextlib import ExitStack

    fdt = mybir.dt.float32
    bdt = mybir.dt.bfloat16
    nc = bacc.Bacc("TRN2", target_bir_lowering=False, debug=False, num_devices=NCORES)
    x_h = nc.declare_dram_parameter("x", [M_LOCAL, K], fdt, isOutput=False)
    w_h = nc.declare_dram_parameter("w", [K, N], fdt, isOutput=False)
    b_h = nc.declare_dram_parameter("bias4", [P, GROUP * N], fdt, isOutput=False)
    y_h = nc.declare_dram_parameter("y", [M_LOCAL, N], fdt, isOutput=True)

    mm_dt = bdt if mode == "bf16" else fdt

    with tile.TileContext(nc) as tc, ExitStack() as ctx:
        xv = x_h.ap().rearrange("(p r) k -> p (r k)", p=P)
        yv = y_h.ap().rearrange("(p r) k -> p (r k)", p=P)

        const = ctx.enter_context(tc.tile_pool(name="const", bufs=1))
        identity = const.tile([P, P], fdt)
        make_identity(nc, identity)
        w_t = const.tile([K, N], fdt)
        nc.sync.dma_start(w_t, w_h.ap())
        bias_t = const.tile([P, GROUP * N], fdt)
        nc.sync.dma_start(bias_t, b_h.ap())
        if mode == "bf16":
            w_mm = const.tile([K, N], bdt)
            nc.vector.tensor_copy(w_mm, w_t)
        else:
            w_mm = w_t

        xin_pool = ctx.enter_context(tc.tile_pool(name="xin", bufs=2))
        yout_pool = ctx.enter_context(tc.tile_pool(name="yout", bufs=2))
        xt_pool = ctx.enter_context(tc.tile_pool(name="xt", bufs=12))
        psT_pool = ctx.enter_context(tc.tile_pool(name="psT", bufs=3, space="PSUM"))
        psM_pool = ctx.enter_context(tc.tile_pool(name="psM", bufs=3, space="PSUM"))

        for _rep in range(reps):
            for c in range(NCHUNK):
                xin = xin_pool.tile([P, CH], fdt)
                nc.sync.dma_start(xin, xv[:, c * CH : (c + 1) * CH])
                yout = yout_pool.tile([P, CH], fdt)
                if mode == "dmaonly":
                    nc.vector.tensor_copy(yout, xin)
                    nc.scalar.dma_start(yv[:, c * CH : (c + 1) * CH], yout)
                    continue
                xts = []
                for g in range(NG):
                    psT = psT_pool.tile([P, GROUP * P], fdt)
                    for i in range(GROUP):
                        r = g * GROUP + i
                        nc.tensor.transpose(
                            psT[:, i * P : (i + 1) * P],
                            xin[:, r * K : (r + 1) * K],
                            identity,
                        )
                    xt = xt_pool.tile([P, GROUP * P], mm_dt)
                    nc.scalar.copy(xt, psT)
                    xts.append(xt)
                if mode == "nomm":
                    for g in range(NG):
                        nc.vector.tensor_add(
                            yout[:, g * GROUP * N : (g + 1) * GROUP * N],
                            xts[g].bitcast(fdt) if mode == "bf16" else xts[g],
                            bias_t,
                        )
                    nc.scalar.dma_start(yv[:, c * CH : (c + 1) * CH], yout)
                    continue
                for g in range(NG):
                    psM = psM_pool.tile([P, GROUP * N], fdt)
                    for i in range(GROUP):
                        nc.tensor.matmul(
                            psM[:, i * N : (i + 1) * N],
                            xts[g][:, i * P : (i + 1) * P],
                            w_mm,
                            start=True,
                            stop=True,
                        )
                    nc.vector.tensor_add(
                        yout[:, g * GROUP * N : (g + 1) * GROUP * N], psM, bias_t
                    )
                nc.scalar.dma_start(yv[:, c * CH : (c + 1) * CH], yout)

    nc.compile()
    _cache[key] = nc
    return nc


def kernel(x, weight, bias, is_training):
    x = np.ascontiguousarray(np.asarray(x, dtype=np.float32))
    weight = np.asarray(weight, dtype=np.float32)
    bias = np.asarray(bias, dtype=np.float32)
    assert x.shape == (M_TOTAL, K), x.shape

    w_bin = _binarize_weight(weight, is_training)
    bias4 = np.ascontiguousarray(np.tile(bias[None, :], (P, GROUP)).astype(np.float32))

    nc = _build()
    from concourse.bass_utils import run_bass_kernel_spmd

    in_maps = [
        {
            "x": x[i * M_LOCAL : (i + 1) * M_LOCAL],
            "w": w_bin,
            "bias4": bias4,
        }
        for i in range(NCORES)
    ]
    res = run_bass_kernel_spmd(nc, in_maps, list(range(NCORES)))
    y = np.concatenate([res.results[i]["y"] for i in range(NCORES)], axis=0)
    return y.astype(np.float32)


# revision 15
# speedup vs baseline: 1.9211x; 1.9211x over previous
"""BinaryDense forward kernel for Trainium2 (8 NeuronCores, data-parallel).

Computes y = x @ w_bin + bias where w_bin is the stochastic binarization
({-1,+1}) of the 128x128 weight (fixed bernoulli key 42, reproduced on host
bit-exactly vs the jax reference; it is a tiny 128x128 computation).

Sharding: x [2097152, 128] f32 is split along M into 8 shards of
[262144, 128], one per NeuronCore; w_bin and bias are replicated.

Per-core kernel (all fp32 — exact wrt the fp32 reference):
  - The shard is processed in 32 chunks of 4 MiB, each a CONTIGUOUS slab of
    8192 rows; SBUF partition p holds slab rows [p*64, (p+1)*64), so every
    HBM<->SBUF DMA is 32 KiB contiguous per partition and the whole transfer
    covers one contiguous DRAM region (maximal HBM row locality — measured
    ~344 GB/s/core combined vs ~200 GB/s with a strided partition map).
  - Input and output DMAs alternate between the two HWDGE rings (SyncE /
    ScalarE issue paths) by chunk parity so their fixed costs overlap.
  - Each [128,128] row-tile (tile r = slab rows {p*64 + r}) is transposed on
    the TensorEngine (transpose mode, exact pass-through, 2 cyc/row fp32)
    into PSUM, evicted to SBUF by ScalarE in groups of 4 tiles per PSUM
    bank, then used as the stationary operand of an fp32 matmul against the
    replicated w_bin (out = xT.T @ w = x @ w, M lands on partitions).
  - y tiles are evicted from PSUM by VectorE with a fused bias add (bias
    pre-tiled on host to [128, 512]) into a staging buffer, DMA'd out with
    the same slab mapping (the permutation is self-inverse).
  - Everything is double/triple buffered; DMA is the bottleneck at the HBM
    roofline (~740 us/core for 128 MiB in + 128 MiB out), PE ~340 us,
    ACT/DVE ~300 us each, all hidden under DMA.
"""

import numpy as np

P = 128  # SBUF partitions
K = 128  # contraction dim
N = 128  # output features
M_TOTAL = 2097152
NCORES = 8
M_LOCAL = M_TOTAL // NCORES  # 262144 rows per core
T = 64  # row-tiles per chunk; chunk = T*128 rows = 4 MiB
NCHUNK = M_LOCAL // (T * P)  # 32 chunks per core
GROUP = 4  # tiles per PSUM bank
NG = T // GROUP  # 16 groups per chunk
CH = T * K  # free elements per chunk per partition

_cache = {}


def _binarize_weight(weight: np.ndarray, is_training) -> np.ndarray:
    """Reproduce the reference's stochastic binarization bit-exactly."""
    training = bool(np.asarray(is_training).item())
    if not training:
        return np.where(weight > 0, 1.0, -1.0).astype(np.float32)
    import jax
    import jax.numpy as jnp

    with jax.default_device(jax.devices("cpu")[0]):
        w = jnp.asarray(weight, dtype=jnp.float32)
        prob_pos = jnp.clip((w + 1.0) / 2.0, 0.0, 1.0)
        bern = jax.random.bernoulli(jax.random.key(42), prob_pos, w.shape)
        w_bin = jnp.where(bern, 1.0, -1.0).astype(jnp.float32)
        return np.asarray(w_bin)


def _build(reps: int = 1):
    """Build + compile the per-core Bass program. `reps` repeats the whole
    workload inside the NEFF (used only for slope benchmarking)."""
    key = ("nc", reps)
    if key in _cache:
        return _cache[key]

    from contextlib import ExitStack

    import concourse.tile as tile
    from concourse import bacc, mybir
    from concourse.masks import make_identity

    fdt = mybir.dt.float32
    nc = bacc.Bacc("TRN2", target_bir_lowering=False, debug=False, num_devices=NCORES)
    x_h = nc.declare_dram_parameter("x", [M_LOCAL, K], fdt, isOutput=False)
    w_h = nc.declare_dram_parameter("w", [K, N], fdt, isOutput=False)
    b_h = nc.declare_dram_parameter("bias4", [P, GROUP * N], fdt, isOutput=False)
    y_h = nc.declare_dram_parameter("y", [M_LOCAL, N], fdt, isOutput=True)

    with tile.TileContext(nc) as tc, ExitStack() as ctx:
        # chunk c is the contiguous slab x[c*8192:(c+1)*8192, :]; partition p
        # holds slab rows [p*64, (p+1)*64) -> 32 KiB contiguous per partition
        xv = x_h.ap().rearrange("(s p r) k -> s p (r k)", p=P, r=T)
        yv = y_h.ap().rearrange("(s p r) k -> s p (r k)", p=P, r=T)

        const = ctx.enter_context(tc.tile_pool(name="const", bufs=1))
        identity = const.tile([P, P], fdt)
        make_identity(nc, identity)
        w_t = const.tile([K, N], fdt)
        nc.sync.dma_start(w_t, w_h.ap())
        bias_t = const.tile([P, GROUP * N], fdt)
        nc.sync.dma_start(bias_t, b_h.ap())

        xin_pool = ctx.enter_context(tc.tile_pool(name="xin", bufs=2))
        yout_pool = ctx.enter_context(tc.tile_pool(name="yout", bufs=2))
        xt_pool = ctx.enter_context(tc.tile_pool(name="xt", bufs=NG + 2))
        psT_pool = ctx.enter_context(tc.tile_pool(name="psT", bufs=3, space="PSUM"))
        psM_pool = ctx.enter_context(tc.tile_pool(name="psM", bufs=3, space="PSUM"))

        for _rep in range(reps):
            for c in range(NCHUNK):
                in_eng = (nc.sync, nc.scalar)[c % 2]
                out_eng = (nc.scalar, nc.sync)[c % 2]

                xin = xin_pool.tile([P, CH], fdt)
                in_eng.dma_start(xin, xv[c])
                yout = yout_pool.tile([P, CH], fdt)

                # phase 1: transpose all tiles of the chunk (PE), evict to
                # SBUF in groups of 4 (ACT)
                xts = []
                for g in range(NG):
                    psT = psT_pool.tile([P, GROUP * P], fdt)
                    for i in range(GROUP):
                        r = g * GROUP + i
                        nc.tensor.transpose(
                            psT[:, i * P : (i + 1) * P],
                            xin[:, r * K : (r + 1) * K],
                            identity,
                        )
                    xt = xt_pool.tile([P, GROUP * P], fdt)
                    nc.scalar.copy(xt, psT)
                    xts.append(xt)

                # phase 2: matmuls (PE), evict + bias (DVE)
                for g in range(NG):
                    psM = psM_pool.tile([P, GROUP * N], fdt)
                    for i in range(GROUP):
                        nc.tensor.matmul(
                            psM[:, i * N : (i + 1) * N],
                            xts[g][:, i * P : (i + 1) * P],
                            w_t,
                            start=True,
                            stop=True,
                        )
                    nc.vector.tensor_add(
                        yout[:, g * GROUP * N : (g + 1) * GROUP * N], psM, bias_t
                    )
                out_eng.dma_start(yv[c], yout)

    nc.compile()
    _cache[key] = nc
    return nc


def kernel(x, weight, bias, is_training):
    x = np.ascontiguousarray(np.asarray(x, dtype=np.float32))
    weight = np.asarray(weight, dtype=np.float32)
    bias = np.asarray(bias, dtype=np.float32)
    assert x.shape == (M_TOTAL, K), x.shape

    w_bin = _binarize_weight(weight, is_training)
    bias4 = np.ascontiguousarray(np.tile(bias[None, :], (P, GROUP)).astype(np.float32))

    nc = _build()
    from concourse.bass_utils import run_bass_kernel_spmd

    in_maps = [
        {
            "x": x[i * M_LOCAL : (i + 1) * M_LOCAL],
            "w": w_bin,
            "bias4": bias4,
        }
        for i in range(NCORES)
    ]
    res = run_bass_kernel_spmd(nc, in_maps, list(range(NCORES)))
    y = np.concatenate([res.results[i]["y"] for i in range(NCORES)], axis=0)
    return y.astype(np.float32)
